# revision 27
# baseline (speedup 1.0000x reference)
"""Trainium2 Bass kernel for nn_Decoder_14894946583396 (dense_mlp).

Reference computation:
    sized = broadcast(representation[B,1,R] -> [B,S,R])   (ones @ rep)
    h     = relu(sized @ W1^T + b1)                       [B,S,HID]
    out   = h @ W2^T + b2                                 [B,S,OUT]

Because every position s within batch b receives the identical input row
representation[b], the MLP output row is identical for all S positions:
    row[b] = relu(rep[b] @ W1^T + b1) @ W2^T + b2         [B,OUT]
    out[b, s, :] = row[b]  for all s

The kernel computes the tiny per-batch MLP on the TensorEngine and
broadcast-writes each row across S with wide SBUF->DRAM DMAs.
Data-parallel across 8 NeuronCores: 4 batches per core, replicated
weights.

Device pipeline per core (fp32 throughout):
  1. two packed input DMAs: pk1 = {x^T, b1, ones, I4, W1^T},
     pk2 = {W2^T, b2}; packing keeps every consumer at one DMA semaphore
     (this walrus allows at most ONE sync wait per instruction).
  2. L1: H[m,h] = x @ W1^T via 8 accumulating matmuls with the tiny x^T
     chunk as the stationary operand (cheap LDWEIGHTS), bias folded in as
     a K=1 ones-matmul, relu on ScalarE.
  3. H -> H^T via 4 PE transposes (H^T needed as stationary for L2).
  4. L2: Y[m,o] = H @ W2^T + b2, 10 matmuls into 2 PSUM banks.
  5. Y rows moved to partition-0 tiles by tiny SBUF->SBUF DMAs (matmul
     operands must start at partition 0/32/64).
  6. Broadcast: K=1 matmul with a ones row as stationary -> [128,512]
     PSUM tiles where every partition holds row[b]; copied 4x along the
     free dim into [128, 4*OUT] SBUF tiles (one writer engine per tile).
  7. 8 output DMAs of 2 MiB each: out[b, 512 s-rows, :] <- tile.

A chain of single-dependency SP nops before the kernel tail makes SP's
vector clock observe every DMA lane and engine, so the final drain needs
no multi-semaphore wait (ISA limit: one sync wait per instruction).
"""

import sys

import numpy as np

if "/opt/trn_rl_repo" not in sys.path:
    sys.path.insert(0, "/opt/trn_rl_repo")

B, S, R = 32, 1024, 1024
HID, OUT = 512, 1024
N_CORES = 8
BPC = B // N_CORES  # batches per core

RC = R // 128  # layer-1 contraction chunks
HC = HID // 128  # layer-2 contraction chunks
OC = OUT // 512  # 512-wide output column chunks

# pk1 column offsets: [p, XTOFF + rc*BPC + m] = rep[m, rc*128+p], etc.
XTOFF = 0
B1OFF = XTOFF + RC * BPC  # row 0: b1
ONOFF = B1OFF + HID  # row 0: 128 ones
I4OFF = ONOFF + 128  # rows 0..3: 4x4 identity
W1OFF = I4OFF + BPC  # [p, W1OFF + rc*HID + h] = W1[h, rc*128+p]
PK1W = W1OFF + RC * HID

# pk2 column offsets
W2OFF = 0  # [p, W2OFF + hc*OUT + o] = W2[o, hc*128+p]
B2OFF = W2OFF + HC * OUT  # row 0: b2
PK2W = B2OFF + OUT

N_COPIES = 4  # row copies along the free dim of each broadcast tile
S_PER_DMA = 128 * N_COPIES  # s-positions covered per output DMA
N_DMAS = S // S_PER_DMA  # output DMAs per batch

_CACHED_NC = None


def _build_nc():
    import concourse.bass as bass
    import concourse.mybir as mybir
    from concourse.tile import TileContext, add_dep_helper

    f32 = mybir.dt.float32
    relu = mybir.ActivationFunctionType.Relu
    fcopy = mybir.ActivationFunctionType.Copy
    nc = bass.Bass()

    pk1 = nc.dram_tensor("pk1", [128, PK1W], f32, kind="ExternalInput")
    pk2 = nc.dram_tensor("pk2", [128, PK2W], f32, kind="ExternalInput")
    out = nc.dram_tensor("out", [BPC, S, OUT], f32, kind="ExternalOutput")

    with TileContext(nc) as tc:
        with (
            tc.tile_pool(name="const", bufs=1) as cpool,
            tc.tile_pool(name="psum_s", bufs=1, space="PSUM") as pp_s,
            tc.tile_pool(name="psum_y", bufs=2, space="PSUM") as pp_y,
            tc.tile_pool(name="psum_t", bufs=1, space="PSUM") as pp_t,
            tc.tile_pool(name="psum_bc", bufs=4, space="PSUM") as pp_bc,
        ):
            # pk1 rides HWDGE lane 0 (fast ~0.6us first byte — it gates all
            # compute). pk2 + row DMAs ride SWDGE lanes; the 8 output DMAs
            # take HWDGE lanes 1..7,0. Only the LAST output trigger reuses a
            # lane (0), and by then its data wait has been observed by the
            # previous trigger, keeping every instruction at <=1 sync wait.
            p1 = cpool.tile([128, PK1W], f32, tag="pk1")
            dma_pk1 = nc.sync.dma_start(out=p1[:, :], in_=pk1[:, :])
            p2 = cpool.tile([128, PK2W], f32, tag="pk2")
            dma_pk2 = nc.gpsimd.dma_start(out=p2[:, :], in_=pk2[:, :])

            # ---- PE warmup: ~4us of dummy matmuls on zeros so the HAM clock
            # gate opens (1.2 -> 2.4 GHz) while pk1 streams in. The warmup
            # shares L1's PSUM tile (slot handoff would emit a non-elidable
            # same-engine wait); L1 then writes rows 0..BPC of it. ----------
            wm_sb = cpool.tile([128, 512], f32, tag="wm")
            nc.vector.memset(wm_sb[:, :], 0.0)
            ph_full = pp_s.tile([128, HID], f32, tag="s")
            for _ in range(10):
                nc.tensor.matmul(
                    ph_full[:, :],
                    lhsT=wm_sb[:, 0:128],
                    rhs=wm_sb[:, :],
                    start=True,
                    stop=True,
                )

            # ---- L1: H[m, h] = x @ W1^T + b1, relu -------------------------
            ph = ph_full[0:BPC, :]
            for rc in range(RC):
                nc.tensor.matmul(
                    ph[:, :],
                    lhsT=p1[:, XTOFF + rc * BPC : XTOFF + (rc + 1) * BPC],
                    rhs=p1[:, W1OFF + rc * HID : W1OFF + rc * HID + HID],
                    start=(rc == 0),
                    stop=False,
                )
            mm_bias1 = nc.tensor.matmul(
                ph[:, :],
                lhsT=p1[0:1, ONOFF : ONOFF + BPC],
                rhs=p1[0:1, B1OFF : B1OFF + HID],
                start=False,
                stop=True,
            )
            # pk2 arrives during L1; observing it here keeps later matmuls
            # at a single sync wait.
            add_dep_helper(mm_bias1.ins, dma_pk2.ins, sync=True, reason="observe pk2")
            h_sb = cpool.tile([BPC, HID], f32, tag="h")
            nc.scalar.activation(h_sb[:, :], ph[:, :], relu)

            # ---- H -> H^T (stationary operand for L2) ----------------------
            ht_sb = cpool.tile([128, HC * BPC], f32, tag="ht")
            for hc in range(HC):
                pt = pp_t.tile([128, BPC], f32, tag="t")
                nc.tensor.transpose(
                    pt[:, :],
                    h_sb[0:BPC, hc * 128 : (hc + 1) * 128],
                    p1[0:BPC, I4OFF : I4OFF + BPC],
                )
                nc.scalar.activation(
                    ht_sb[:, hc * BPC : (hc + 1) * BPC], pt[:, :], fcopy
                )

            # ---- L2: Y[m, o] = H @ W2^T + b2 -------------------------------
            y_sb = cpool.tile([BPC, OUT], f32, tag="y")
            for oc in range(OC):
                py = pp_y.tile([BPC, 512], f32, tag="y")
                for hc in range(HC):
                    nc.tensor.matmul(
                        py[:, :],
                        lhsT=ht_sb[:, hc * BPC : (hc + 1) * BPC],
                        rhs=p2[
                            :, W2OFF + hc * OUT + oc * 512 : W2OFF + hc * OUT + oc * 512 + 512
                        ],
                        start=(hc == 0),
                        stop=False,
                    )
                nc.tensor.matmul(
                    py[:, :],
                    lhsT=p1[0:1, ONOFF : ONOFF + BPC],
                    rhs=p2[0:1, B2OFF + oc * 512 : B2OFF + (oc + 1) * 512],
                    start=False,
                    stop=True,
                )
                nc.vector.tensor_copy(y_sb[:, oc * 512 : (oc + 1) * 512], py[:, :])

            # ---- move Y rows to partition 0 (matmul base-partition rule) ---
            yrows = []
            row_dmas = []
            for b in range(BPC):
                yr = cpool.tile([1, OUT], f32, tag=f"yr{b}")
                d = nc.gpsimd.dma_start(out=yr[0:1, :], in_=y_sb[b : b + 1, :])
                yrows.append(yr)
                row_dmas.append(d)

            # ---- broadcast rows across partitions, replicate, store --------
            out_dmas = []
            bc_idx = 0
            for b in range(BPC):
                yt = cpool.tile([128, N_COPIES * OUT], f32, tag=f"yt{b}")
                copy_eng = "dve" if b % 2 == 0 else "act"
                for oc in range(OC):
                    pb = pp_bc.tile([128, 512], f32, tag="bc")
                    mm = nc.tensor.matmul(
                        pb[:, :],
                        lhsT=p1[0:1, ONOFF : ONOFF + 128],
                        rhs=yrows[b][0:1, oc * 512 : (oc + 1) * 512],
                        start=True,
                        stop=True,
                    )
                    # Greedy lane observation: matmuls 1-3 have no natural
                    # lane wait, so each observes the next row-DMA's lane;
                    # later ones then only wait on their PSUM slot release.
                    if 1 <= bc_idx <= BPC - 1:
                        add_dep_helper(
                            mm.ins,
                            row_dmas[bc_idx].ins,
                            sync=True,
                            reason="observe next yrow lane",
                        )
                    bc_idx += 1
                    last_mm = mm
                    # PSUM -> SBUF once per oc half...
                    dst = yt[:, oc * 512 : (oc + 1) * 512]
                    if copy_eng == "dve":
                        last_dve = nc.vector.tensor_copy(dst, pb[:, :])
                    else:
                        last_act = nc.scalar.activation(dst, pb[:, :], fcopy)
                # ...then replicate with wide SBUF->SBUF copies (2x f32 mode)
                for c in range(1, N_COPIES):
                    dst = yt[:, c * OUT : (c + 1) * OUT]
                    if copy_eng == "dve":
                        last_dve = nc.vector.tensor_copy(dst, yt[:, 0:OUT])
                    else:
                        last_act = nc.scalar.activation(dst, yt[:, 0:OUT], fcopy)
                # each DMA writes S_PER_DMA consecutive s rows (all identical)
                for j in range(N_DMAS):
                    d = nc.sync.dma_start(
                        out=out[b, j * S_PER_DMA : (j + 1) * S_PER_DMA, :].rearrange(
                            "(p c) o -> p c o", c=N_COPIES
                        ),
                        in_=yt[:, :].rearrange("p (c o) -> p c o", o=OUT),
                    )
                    out_dmas.append(d)

            # The kernel-tail drain waits on every proc's final tick, but this
            # walrus allows at most ONE sync wait per instruction. Chain SP
            # nops, one dependency each, so SP's vector clock observes the
            # final tick of every DMA lane and engine before the drain.
            tail = out_dmas + [dma_pk1, dma_pk2] + row_dmas + [last_mm, last_act, last_dve]
            for d in tail:
                n = nc.sync.nop(nofuse=True)
                add_dep_helper(
                    n.ins, d.ins, sync=True, reason="observe final ticks pre-drain"
                )

    return nc


def _get_nc():
    global _CACHED_NC
    if _CACHED_NC is None:
        _CACHED_NC = _build_nc()
    return _CACHED_NC


def _prep_in_maps(representation, W1, b1, W2, b2):
    rep = np.asarray(representation, dtype=np.float32).reshape(B, R)
    w1 = np.asarray(W1, dtype=np.float32)
    w2 = np.asarray(W2, dtype=np.float32)
    b1 = np.asarray(b1, dtype=np.float32)
    b2 = np.asarray(b2, dtype=np.float32)

    base1 = np.zeros((128, PK1W), dtype=np.float32)
    base1[0, B1OFF : B1OFF + HID] = b1
    base1[0, ONOFF : ONOFF + 128] = 1.0
    base1[0:BPC, I4OFF : I4OFF + BPC] = np.eye(BPC, dtype=np.float32)
    base1[:, W1OFF : W1OFF + RC * HID] = (
        w1.T.reshape(RC, 128, HID).transpose(1, 0, 2).reshape(128, RC * HID)
    )

    pk2 = np.zeros((128, PK2W), dtype=np.float32)
    pk2[:, W2OFF : W2OFF + HC * OUT] = (
        w2.T.reshape(HC, 128, OUT).transpose(1, 0, 2).reshape(128, HC * OUT)
    )
    pk2[0, B2OFF : B2OFF + OUT] = b2

    in_maps = []
    for c in range(N_CORES):
        xt = rep[c * BPC : (c + 1) * BPC].T  # [R, BPC]
        pk1 = base1.copy()
        pk1[:, XTOFF : XTOFF + RC * BPC] = (
            xt.reshape(RC, 128, BPC).transpose(1, 0, 2).reshape(128, RC * BPC)
        )
        in_maps.append({"pk1": pk1, "pk2": pk2})
    return in_maps


def run_sharded(representation, W1, b1, W2, b2, **run_kwargs):
    """Compile+run on 8 cores; returns (full_output, BassKernelResults)."""
    from concourse.bass_utils import run_bass_kernel_spmd

    nc = _get_nc()
    in_maps = _prep_in_maps(representation, W1, b1, W2, b2)
    res = run_bass_kernel_spmd(nc, in_maps, core_ids=list(range(N_CORES)), **run_kwargs)
    full = np.concatenate([r["out"] for r in res.results], axis=0)
    return full, res


def kernel(representation, size_matrix=None, W1=None, b1=None, W2=None, b2=None):
    # size_matrix only contributes its shape in the reference (ones_like);
    # its values are unused.
    full, _ = run_sharded(representation, W1, b1, W2, b2)
    return full


# revision 28
# speedup vs baseline: 1.0982x; 1.0982x over previous
"""Trainium2 Bass kernel for nn_Decoder_14894946583396 (dense_mlp).

Reference computation:
    sized = broadcast(representation[B,1,R] -> [B,S,R])   (ones @ rep)
    h     = relu(sized @ W1^T + b1)                       [B,S,HID]
    out   = h @ W2^T + b2                                 [B,S,OUT]

Because every position s within batch b receives the identical input row
representation[b], the MLP output row is identical for all S positions:
    row[b] = relu(rep[b] @ W1^T + b1) @ W2^T + b2         [B,OUT]
    out[b, s, :] = row[b]  for all s

The kernel computes the tiny per-batch MLP on the TensorEngine (fp32,
bit-exact vs the f32 reference) and broadcast-writes each row across S
with wide SBUF->DRAM DMAs. Data-parallel across 8 NeuronCores: 4 batches
per core, replicated weights.

Device pipeline per core:
  1. Four input DMAs: pk1a = {x^T, I4} (tiny, HWDGE lane 0 — it gates
     compute), prow = {b1, ones, b2} single row, w1 = W1^T, w2 = W2^T
     (all three on SWDGE lanes, streaming under the warmup).
  2. ~10 us of dummy matmuls on zeros warm the PE HAM clock gate
     (1.2 -> 2.4 GHz) while weights stream in.
  3. L1: H[m,h] = x @ W1^T via 8 accumulating matmuls with the tiny x^T
     chunk as stationary (cheap LDWEIGHTS), bias folded in as a K=1
     ones-matmul, relu on ScalarE.
  4. H -> H^T via 4 PE transposes (stationary operand for L2).
  5. L2: Y[m,o] = H @ W2^T + b2, 10 matmuls into 2 PSUM banks.
  6. Y rows moved to partition-0 tiles by tiny SBUF->SBUF DMAs (matmul
     operands must start at partition 0/32/64).
  7. Broadcast: K=1 matmul with a ones row as stationary -> [128,512]
     PSUM tiles where every partition holds row[b]; one PSUM->SBUF copy
     per half, then wide SBUF->SBUF replication copies (one writer
     engine per tile).
  8. 8 output DMAs of 2 MiB each on the 8 fresh HWDGE lanes.

Single-sync-wait discipline (this walrus rejects 2+ waits on any
instruction): inputs are packed so every consumer sees one DMA
semaphore; SWDGE lanes carry everything but pk1a and the outputs so no
HWDGE output trigger reuses a lane whose data wait is unobserved;
artificial add_dep_helper edges pre-observe upcoming DMA lanes on
instructions that have a free wait slot; and a chain of 1-wait SP nops
before the TileContext exit drain leaves the drain with nothing to wait
on.
"""

import sys

import numpy as np

if "/opt/trn_rl_repo" not in sys.path:
    sys.path.insert(0, "/opt/trn_rl_repo")

B, S, R = 32, 1024, 1024
HID, OUT = 512, 1024
N_CORES = 8
BPC = B // N_CORES  # batches per core

RC = R // 128  # layer-1 contraction chunks
HC = HID // 128  # layer-2 contraction chunks
OC = OUT // 512  # 512-wide output column chunks

# pk1a columns: [p, rc*BPC + m] = rep[m, rc*128+p], then a 4x4 identity
XTOFF = 0
I4OFF = XTOFF + RC * BPC
PK1AW = I4OFF + BPC
# prow columns (single partition row)
B1OFF = 0
ONOFF = B1OFF + HID
B2OFF = ONOFF + 128
PROWW = B2OFF + OUT
# w1: [p, rc*HID + h] = W1[h, rc*128+p];  w2: [p, hc*OUT + o] = W2[o, hc*128+p]

N_COPIES = 4  # row copies along the free dim of each broadcast tile
S_PER_DMA = 128 * N_COPIES  # s-positions covered per output DMA
N_DMAS = S // S_PER_DMA  # output DMAs per batch
N_WARMUP = 8

_CACHED_NC = None


def _build_nc():
    import concourse.bass as bass
    import concourse.mybir as mybir
    from concourse.tile import TileContext, add_dep_helper

    f32 = mybir.dt.float32
    relu = mybir.ActivationFunctionType.Relu
    fcopy = mybir.ActivationFunctionType.Copy
    nc = bass.Bass()

    pk1a = nc.dram_tensor("pk1a", [128, PK1AW], f32, kind="ExternalInput")
    prow = nc.dram_tensor("prow", [1, PROWW], f32, kind="ExternalInput")
    w1 = nc.dram_tensor("w1", [128, RC * HID], f32, kind="ExternalInput")
    w2 = nc.dram_tensor("w2", [128, HC * OUT], f32, kind="ExternalInput")
    out = nc.dram_tensor("out", [BPC, S, OUT], f32, kind="ExternalOutput")

    with TileContext(nc) as tc:
        with (
            tc.tile_pool(name="const", bufs=1) as cpool,
            tc.tile_pool(name="psum_s", bufs=1, space="PSUM") as pp_s,
            tc.tile_pool(name="psum_y", bufs=2, space="PSUM") as pp_y,
            tc.tile_pool(name="psum_t", bufs=1, space="PSUM") as pp_t,
            tc.tile_pool(name="psum_bc", bufs=4, space="PSUM") as pp_bc,
        ):
            p1a = cpool.tile([128, PK1AW], f32, tag="pk1a")
            nc.sync.dma_start(out=p1a[:, :], in_=pk1a[:, :])
            prow_sb = cpool.tile([1, PROWW], f32, tag="prow")
            dma_prow = nc.gpsimd.dma_start(out=prow_sb[0:1, :], in_=prow[0:1, :])
            w1_sb = cpool.tile([128, RC * HID], f32, tag="w1")
            dma_w1 = nc.gpsimd.dma_start(out=w1_sb[:, :], in_=w1[:, :])
            w2_sb = cpool.tile([128, HC * OUT], f32, tag="w2")
            dma_w2 = nc.gpsimd.dma_start(out=w2_sb[:, :], in_=w2[:, :])

            # ---- PE warmup on zeros; shares L1's PSUM tile (a slot handoff
            # would emit a non-elidable same-engine wait) -------------------
            wm_sb = cpool.tile([128, 512], f32, tag="wm")
            nc.vector.memset(wm_sb[:, :], 0.0)
            ph_full = pp_s.tile([128, HID], f32, tag="s")
            for k in range(N_WARMUP):
                wmm = nc.tensor.matmul(
                    ph_full[:, :],
                    lhsT=wm_sb[:, 0:128],
                    rhs=wm_sb[:, :],
                    start=True,
                    stop=True,
                )
            # the last warmup matmul observes w1's lane so L1's first matmul
            # only needs the pk1a wait
            add_dep_helper(wmm.ins, dma_w1.ins, sync=True, reason="observe w1")

            # ---- L1: H[m, h] = x @ W1^T + b1, relu -------------------------
            ph = ph_full[0:BPC, :]
            for rc in range(RC):
                mm = nc.tensor.matmul(
                    ph[:, :],
                    lhsT=p1a[:, XTOFF + rc * BPC : XTOFF + (rc + 1) * BPC],
                    rhs=w1_sb[:, rc * HID : rc * HID + HID],
                    start=(rc == 0),
                    stop=False,
                )
            # rc=7 has a free wait slot: pre-observe w2's lane for L2
            add_dep_helper(mm.ins, dma_w2.ins, sync=True, reason="observe w2")
            nc.tensor.matmul(
                ph[:, :],
                lhsT=prow_sb[0:1, ONOFF : ONOFF + BPC],
                rhs=prow_sb[0:1, B1OFF : B1OFF + HID],
                start=False,
                stop=True,
            )
            h_sb = cpool.tile([BPC, HID], f32, tag="h")
            nc.scalar.activation(h_sb[:, :], ph[:, :], relu)

            # ---- H -> H^T (stationary operand for L2) ----------------------
            ht_sb = cpool.tile([128, HC * BPC], f32, tag="ht")
            for hc in range(HC):
                pt = pp_t.tile([128, BPC], f32, tag="t")
                nc.tensor.transpose(
                    pt[:, :],
                    h_sb[0:BPC, hc * 128 : (hc + 1) * 128],
                    p1a[0:BPC, I4OFF : I4OFF + BPC],
                )
                nc.scalar.activation(
                    ht_sb[:, hc * BPC : (hc + 1) * BPC], pt[:, :], fcopy
                )

            # ---- L2: Y[m, o] = H @ W2^T + b2 -------------------------------
            y_sb = cpool.tile([BPC, OUT], f32, tag="y")
            for oc in range(OC):
                py = pp_y.tile([BPC, 512], f32, tag="y")
                for hc in range(HC):
                    nc.tensor.matmul(
                        py[:, :],
                        lhsT=ht_sb[:, hc * BPC : (hc + 1) * BPC],
                        rhs=w2_sb[:, hc * OUT + oc * 512 : hc * OUT + oc * 512 + 512],
                        start=(hc == 0),
                        stop=False,
                    )
                nc.tensor.matmul(
                    py[:, :],
                    lhsT=prow_sb[0:1, ONOFF : ONOFF + BPC],
                    rhs=prow_sb[0:1, B2OFF + oc * 512 : B2OFF + (oc + 1) * 512],
                    start=False,
                    stop=True,
                )
                nc.vector.tensor_copy(y_sb[:, oc * 512 : (oc + 1) * 512], py[:, :])

            # ---- move Y rows to partition 0 (matmul base-partition rule) ---
            yrows = []
            row_dmas = []
            for b in range(BPC):
                yr = cpool.tile([1, OUT], f32, tag=f"yr{b}")
                d = nc.gpsimd.dma_start(out=yr[0:1, :], in_=y_sb[b : b + 1, :])
                yrows.append(yr)
                row_dmas.append(d)

            # ---- broadcast rows across partitions, replicate, store --------
            out_dmas = []
            bc_idx = 0
            for b in range(BPC):
                yt = cpool.tile([128, N_COPIES * OUT], f32, tag=f"yt{b}")
                copy_eng = "dve" if b % 2 == 0 else "act"
                for oc in range(OC):
                    pb = pp_bc.tile([128, 512], f32, tag="bc")
                    mm = nc.tensor.matmul(
                        pb[:, :],
                        lhsT=prow_sb[0:1, ONOFF : ONOFF + 128],
                        rhs=yrows[b][0:1, oc * 512 : (oc + 1) * 512],
                        start=True,
                        stop=True,
                    )
                    # Greedy lane observation: matmuls 1-3 have no natural
                    # lane wait, so each observes the next row-DMA's lane;
                    # later ones then only wait on their PSUM slot release.
                    if 1 <= bc_idx <= BPC - 1:
                        add_dep_helper(
                            mm.ins,
                            row_dmas[bc_idx].ins,
                            sync=True,
                            reason="observe next yrow lane",
                        )
                    bc_idx += 1
                    last_mm = mm
                    # PSUM -> SBUF once per oc half...
                    dst = yt[:, oc * 512 : (oc + 1) * 512]
                    if copy_eng == "dve":
                        last_dve = nc.vector.tensor_copy(dst, pb[:, :])
                    else:
                        last_act = nc.scalar.activation(dst, pb[:, :], fcopy)
                # ...then replicate with wide SBUF->SBUF copies (2x f32 mode)
                for c in range(1, N_COPIES):
                    dst = yt[:, c * OUT : (c + 1) * OUT]
                    if copy_eng == "dve":
                        last_dve = nc.vector.tensor_copy(dst, yt[:, 0:OUT])
                    else:
                        last_act = nc.scalar.activation(dst, yt[:, 0:OUT], fcopy)
                # each DMA writes S_PER_DMA consecutive s rows (all identical)
                for j in range(N_DMAS):
                    d = nc.sync.dma_start(
                        out=out[b, j * S_PER_DMA : (j + 1) * S_PER_DMA, :].rearrange(
                            "(p c) o -> p c o", c=N_COPIES
                        ),
                        in_=yt[:, :].rearrange("p (c o) -> p c o", o=OUT),
                    )
                    out_dmas.append(d)

            # The kernel-tail drain waits on every proc's final tick, but this
            # walrus allows at most ONE sync wait per instruction. Chain SP
            # nops, one dependency each, so SP's vector clock observes the
            # final tick of every DMA lane and engine before the drain.
            tail = (
                out_dmas
                + [dma_prow, dma_w1, dma_w2]
                + row_dmas
                + [last_mm, last_act, last_dve]
            )
            for d in tail:
                n = nc.sync.nop(nofuse=True)
                add_dep_helper(
                    n.ins, d.ins, sync=True, reason="observe final ticks pre-drain"
                )

    return nc


def _get_nc():
    global _CACHED_NC
    if _CACHED_NC is None:
        _CACHED_NC = _build_nc()
    return _CACHED_NC


def _prep_in_maps(representation, W1, b1, W2, b2):
    rep = np.asarray(representation, dtype=np.float32).reshape(B, R)
    w1 = np.asarray(W1, dtype=np.float32)
    w2 = np.asarray(W2, dtype=np.float32)
    b1 = np.asarray(b1, dtype=np.float32)
    b2 = np.asarray(b2, dtype=np.float32)

    w1p = np.ascontiguousarray(
        w1.T.reshape(RC, 128, HID).transpose(1, 0, 2).reshape(128, RC * HID)
    )
    w2p = np.ascontiguousarray(
        w2.T.reshape(HC, 128, OUT).transpose(1, 0, 2).reshape(128, HC * OUT)
    )
    prow = np.zeros((1, PROWW), dtype=np.float32)
    prow[0, B1OFF : B1OFF + HID] = b1
    prow[0, ONOFF : ONOFF + 128] = 1.0
    prow[0, B2OFF : B2OFF + OUT] = b2

    in_maps = []
    for c in range(N_CORES):
        xt = rep[c * BPC : (c + 1) * BPC].T  # [R, BPC]
        pk1a = np.zeros((128, PK1AW), dtype=np.float32)
        pk1a[:, XTOFF : XTOFF + RC * BPC] = (
            xt.reshape(RC, 128, BPC).transpose(1, 0, 2).reshape(128, RC * BPC)
        )
        pk1a[0:BPC, I4OFF : I4OFF + BPC] = np.eye(BPC, dtype=np.float32)
        in_maps.append({"pk1a": pk1a, "prow": prow, "w1": w1p, "w2": w2p})
    return in_maps


def run_sharded(representation, W1, b1, W2, b2, **run_kwargs):
    """Compile+run on 8 cores; returns (full_output, BassKernelResults)."""
    from concourse.bass_utils import run_bass_kernel_spmd

    nc = _get_nc()
    in_maps = _prep_in_maps(representation, W1, b1, W2, b2)
    res = run_bass_kernel_spmd(nc, in_maps, core_ids=list(range(N_CORES)), **run_kwargs)
    full = np.concatenate([r["out"] for r in res.results], axis=0)
    return full, res


def kernel(representation, size_matrix=None, W1=None, b1=None, W2=None, b2=None):
    # size_matrix only contributes its shape in the reference (ones_like);
    # its values are unused.
    full, _ = run_sharded(representation, W1, b1, W2, b2)
    return full


# revision 32
# speedup vs baseline: 1.2808x; 1.1663x over previous
"""Trainium2 Bass kernel for nn_Decoder_14894946583396 (dense_mlp).

Reference computation:
    sized = broadcast(representation[B,1,R] -> [B,S,R])   (ones @ rep)
    h     = relu(sized @ W1^T + b1)                       [B,S,HID]
    out   = h @ W2^T + b2                                 [B,S,OUT]

Because every position s within batch b receives the identical input row
representation[b], the MLP output row is identical for all S positions:
    row[b] = relu(rep[b] @ W1^T + b1) @ W2^T + b2         [B,OUT]
    out[b, s, :] = row[b]  for all s

The kernel computes the tiny per-batch MLP on the TensorEngine (fp32,
bit-exact vs the f32 reference) and broadcast-writes each row across S
with wide SBUF->DRAM DMAs. Data-parallel across 8 NeuronCores: 4 batches
per core, replicated weights.

Device pipeline per core:
  1. Four input DMAs: pk1a = {x^T, I4} (tiny, HWDGE lane 0 — it gates
     compute), prow = {b1, ones, b2} single row, w1 = W1^T, w2 = W2^T
     (all three on SWDGE lanes, streaming under the warmup).
  2. ~10 us of dummy matmuls on zeros warm the PE HAM clock gate
     (1.2 -> 2.4 GHz) while weights stream in.
  3. L1: H[m,h] = x @ W1^T via 8 accumulating matmuls with the tiny x^T
     chunk as stationary (cheap LDWEIGHTS), bias folded in as a K=1
     ones-matmul, relu on ScalarE.
  4. H -> H^T via 4 PE transposes (stationary operand for L2).
  5. L2: Y[m,o] = H @ W2^T + b2, 10 matmuls into 2 PSUM banks.
  6. Y rows moved to partition-0 tiles by tiny SBUF->SBUF DMAs (matmul
     operands must start at partition 0/32/64).
  7. Broadcast: K=1 matmul with a ones row as stationary -> [128,512]
     PSUM tiles where every partition holds row[b]; one PSUM->SBUF copy
     per half, then wide SBUF->SBUF replication copies (one writer
     engine per tile).
  8. 8 output DMAs of 2 MiB each on the 8 fresh HWDGE lanes.

Single-sync-wait discipline (this walrus rejects 2+ waits on any
instruction): inputs are packed so every consumer sees one DMA
semaphore; SWDGE lanes carry everything but pk1a and the outputs so no
HWDGE output trigger reuses a lane whose data wait is unobserved;
artificial add_dep_helper edges pre-observe upcoming DMA lanes on
instructions that have a free wait slot; and a chain of 1-wait SP nops
before the TileContext exit drain leaves the drain with nothing to wait
on.
"""

import sys

import numpy as np

if "/opt/trn_rl_repo" not in sys.path:
    sys.path.insert(0, "/opt/trn_rl_repo")

B, S, R = 32, 1024, 1024
HID, OUT = 512, 1024
N_CORES = 8
BPC = B // N_CORES  # batches per core

RC = R // 128  # layer-1 contraction chunks
HC = HID // 128  # layer-2 contraction chunks
OC = OUT // 512  # 512-wide output column chunks

# pk1a columns: [p, rc*BPC + m] = rep[m, rc*128+p], then a 4x4 identity,
# then 4 selector-broadcast blocks: [k, SELOFF + b*128 + m] = (k == b)
XTOFF = 0
I4OFF = XTOFF + RC * BPC
SELOFF = I4OFF + BPC
PK1AW = SELOFF + BPC * 128
# prow columns (single partition row)
B1OFF = 0
ONOFF = B1OFF + HID
B2OFF = ONOFF + 128
PROWW = B2OFF + OUT
# w1: [p, rc*HID + h] = W1[h, rc*128+p];  w2: [p, hc*OUT + o] = W2[o, hc*128+p]

N_COPIES = 4  # row copies along the free dim of each broadcast tile
S_PER_DMA = 128 * N_COPIES  # s-positions covered per output DMA
N_DMAS = S // S_PER_DMA  # output DMAs per batch
N_WARMUP = 8

_CACHED_NC = None


def _build_nc():
    import concourse.bass as bass
    import concourse.mybir as mybir
    from concourse.tile import TileContext, add_dep_helper

    f32 = mybir.dt.float32
    relu = mybir.ActivationFunctionType.Relu
    fcopy = mybir.ActivationFunctionType.Copy
    nc = bass.Bass()

    pk1a = nc.dram_tensor("pk1a", [128, PK1AW], f32, kind="ExternalInput")
    prow = nc.dram_tensor("prow", [1, PROWW], f32, kind="ExternalInput")
    w1 = nc.dram_tensor("w1", [128, RC * HID], f32, kind="ExternalInput")
    w2 = nc.dram_tensor("w2", [128, HC * OUT], f32, kind="ExternalInput")
    out = nc.dram_tensor("out", [BPC, S, OUT], f32, kind="ExternalOutput")

    with TileContext(nc) as tc:
        with (
            tc.tile_pool(name="const", bufs=1) as cpool,
            tc.tile_pool(name="psum_s", bufs=1, space="PSUM") as pp_s,
            tc.tile_pool(name="psum_y", bufs=2, space="PSUM") as pp_y,
            tc.tile_pool(name="psum_t", bufs=1, space="PSUM") as pp_t,
            tc.tile_pool(name="psum_bc", bufs=4, space="PSUM") as pp_bc,
        ):
            p1a = cpool.tile([128, PK1AW], f32, tag="pk1a")
            nc.sync.dma_start(out=p1a[:, :], in_=pk1a[:, :])
            prow_sb = cpool.tile([1, PROWW], f32, tag="prow")
            dma_prow = nc.gpsimd.dma_start(out=prow_sb[0:1, :], in_=prow[0:1, :])
            w1_sb = cpool.tile([128, RC * HID], f32, tag="w1")
            dma_w1 = nc.gpsimd.dma_start(out=w1_sb[:, :], in_=w1[:, :])
            w2_sb = cpool.tile([128, HC * OUT], f32, tag="w2")
            dma_w2 = nc.gpsimd.dma_start(out=w2_sb[:, :], in_=w2[:, :])

            # ---- PE warmup on zeros; shares L1's PSUM tile (a slot handoff
            # would emit a non-elidable same-engine wait) -------------------
            wm_sb = cpool.tile([128, 512], f32, tag="wm")
            nc.vector.memset(wm_sb[:, :], 0.0)
            ph_full = pp_s.tile([128, HID], f32, tag="s")
            for k in range(N_WARMUP):
                wmm = nc.tensor.matmul(
                    ph_full[:, :],
                    lhsT=wm_sb[:, 0:128],
                    rhs=wm_sb[:, :],
                    start=True,
                    stop=True,
                )
            # the last warmup matmul observes w1's lane so L1's first matmul
            # only needs the pk1a wait
            add_dep_helper(wmm.ins, dma_w1.ins, sync=True, reason="observe w1")

            # ---- L1: H[m, h] = x @ W1^T + b1, relu -------------------------
            ph = ph_full[0:BPC, :]
            for rc in range(RC):
                mm = nc.tensor.matmul(
                    ph[:, :],
                    lhsT=p1a[:, XTOFF + rc * BPC : XTOFF + (rc + 1) * BPC],
                    rhs=w1_sb[:, rc * HID : rc * HID + HID],
                    start=(rc == 0),
                    stop=False,
                )
            # rc=7 has a free wait slot: pre-observe w2's lane for L2
            add_dep_helper(mm.ins, dma_w2.ins, sync=True, reason="observe w2")
            nc.tensor.matmul(
                ph[:, :],
                lhsT=prow_sb[0:1, ONOFF : ONOFF + BPC],
                rhs=prow_sb[0:1, B1OFF : B1OFF + HID],
                start=False,
                stop=True,
            )
            h_sb = cpool.tile([BPC, HID], f32, tag="h")
            nc.scalar.activation(h_sb[:, :], ph[:, :], relu)

            # ---- H -> H^T (stationary operand for L2) ----------------------
            ht_sb = cpool.tile([128, HC * BPC], f32, tag="ht")
            for hc in range(HC):
                pt = pp_t.tile([128, BPC], f32, tag="t")
                nc.tensor.transpose(
                    pt[:, :],
                    h_sb[0:BPC, hc * 128 : (hc + 1) * 128],
                    p1a[0:BPC, I4OFF : I4OFF + BPC],
                )
                nc.scalar.activation(
                    ht_sb[:, hc * BPC : (hc + 1) * BPC], pt[:, :], fcopy
                )

            # ---- L2: Y[m, o] = H @ W2^T + b2 -------------------------------
            # per-oc Y tiles so the broadcast of the first half can start
            # while the second half's matmuls still run
            y_halves = []
            for oc in range(OC):
                py = pp_y.tile([BPC, 512], f32, tag="y")
                for hc in range(HC):
                    nc.tensor.matmul(
                        py[:, :],
                        lhsT=ht_sb[:, hc * BPC : (hc + 1) * BPC],
                        rhs=w2_sb[:, hc * OUT + oc * 512 : hc * OUT + oc * 512 + 512],
                        start=(hc == 0),
                        stop=False,
                    )
                nc.tensor.matmul(
                    py[:, :],
                    lhsT=prow_sb[0:1, ONOFF : ONOFF + BPC],
                    rhs=prow_sb[0:1, B2OFF + oc * 512 : B2OFF + (oc + 1) * 512],
                    start=False,
                    stop=True,
                )
                yh = cpool.tile([BPC, 512], f32, tag=f"yh{oc}")
                nc.vector.tensor_copy(yh[:, :], py[:, :])
                y_halves.append(yh)

            # ---- broadcast rows across partitions, replicate, store --------
            # A K=4 selector matmul (lhsT = e_b outer ones, host-packed)
            # extracts row b of Y AND replicates it across all 128 output
            # partitions in one PE op — both operands at base partition 0.
            out_dmas = []
            for b in range(BPC):
                yt = cpool.tile([128, N_COPIES * OUT], f32, tag=f"yt{b}")
                copy_eng = "dve" if b % 2 == 0 else "act"
                for oc in range(OC):
                    pb = pp_bc.tile([128, 512], f32, tag="bc")
                    mm = nc.tensor.matmul(
                        pb[:, :],
                        lhsT=p1a[0:BPC, SELOFF + b * 128 : SELOFF + (b + 1) * 128],
                        rhs=y_halves[oc][0:BPC, :],
                        start=True,
                        stop=True,
                    )
                    last_mm = mm
                    # PSUM -> SBUF once per oc half...
                    dst = yt[:, oc * 512 : (oc + 1) * 512]
                    if copy_eng == "dve":
                        last_dve = nc.vector.tensor_copy(dst, pb[:, :])
                    else:
                        last_act = nc.scalar.activation(dst, pb[:, :], fcopy)
                # ...then replicate with wide SBUF->SBUF copies (2x f32 mode)
                for c in range(1, N_COPIES):
                    dst = yt[:, c * OUT : (c + 1) * OUT]
                    if copy_eng == "dve":
                        last_dve = nc.vector.tensor_copy(dst, yt[:, 0:OUT])
                    else:
                        last_act = nc.scalar.activation(dst, yt[:, 0:OUT], fcopy)
                # each DMA writes S_PER_DMA consecutive s rows (all identical)
                for j in range(N_DMAS):
                    d = nc.sync.dma_start(
                        out=out[b, j * S_PER_DMA : (j + 1) * S_PER_DMA, :].rearrange(
                            "(p c) o -> p c o", c=N_COPIES
                        ),
                        in_=yt[:, :].rearrange("p (c o) -> p c o", o=OUT),
                    )
                    out_dmas.append(d)

            # The kernel-tail drain waits on every proc's final tick, but this
            # walrus allows at most ONE sync wait per instruction. Chain SP
            # nops, one dependency each, so SP's vector clock observes the
            # final tick of every DMA lane and engine before the drain.
            tail = out_dmas + [dma_prow, dma_w1, dma_w2, last_mm, last_act, last_dve]
            for d in tail:
                n = nc.sync.nop(nofuse=True)
                add_dep_helper(
                    n.ins, d.ins, sync=True, reason="observe final ticks pre-drain"
                )

    return nc


def _get_nc():
    global _CACHED_NC
    if _CACHED_NC is None:
        _CACHED_NC = _build_nc()
    return _CACHED_NC


def _prep_in_maps(representation, W1, b1, W2, b2):
    rep = np.asarray(representation, dtype=np.float32).reshape(B, R)
    w1 = np.asarray(W1, dtype=np.float32)
    w2 = np.asarray(W2, dtype=np.float32)
    b1 = np.asarray(b1, dtype=np.float32)
    b2 = np.asarray(b2, dtype=np.float32)

    w1p = np.ascontiguousarray(
        w1.T.reshape(RC, 128, HID).transpose(1, 0, 2).reshape(128, RC * HID)
    )
    w2p = np.ascontiguousarray(
        w2.T.reshape(HC, 128, OUT).transpose(1, 0, 2).reshape(128, HC * OUT)
    )
    prow = np.zeros((1, PROWW), dtype=np.float32)
    prow[0, B1OFF : B1OFF + HID] = b1
    prow[0, ONOFF : ONOFF + 128] = 1.0
    prow[0, B2OFF : B2OFF + OUT] = b2

    in_maps = []
    for c in range(N_CORES):
        xt = rep[c * BPC : (c + 1) * BPC].T  # [R, BPC]
        pk1a = np.zeros((128, PK1AW), dtype=np.float32)
        pk1a[:, XTOFF : XTOFF + RC * BPC] = (
            xt.reshape(RC, 128, BPC).transpose(1, 0, 2).reshape(128, RC * BPC)
        )
        pk1a[0:BPC, I4OFF : I4OFF + BPC] = np.eye(BPC, dtype=np.float32)
        for b in range(BPC):
            pk1a[b, SELOFF + b * 128 : SELOFF + (b + 1) * 128] = 1.0
        in_maps.append({"pk1a": pk1a, "prow": prow, "w1": w1p, "w2": w2p})
    return in_maps


def run_sharded(representation, W1, b1, W2, b2, **run_kwargs):
    """Compile+run on 8 cores; returns (full_output, BassKernelResults)."""
    from concourse.bass_utils import run_bass_kernel_spmd

    nc = _get_nc()
    in_maps = _prep_in_maps(representation, W1, b1, W2, b2)
    res = run_bass_kernel_spmd(nc, in_maps, core_ids=list(range(N_CORES)), **run_kwargs)
    full = np.concatenate([r["out"] for r in res.results], axis=0)
    return full, res


def kernel(representation, size_matrix=None, W1=None, b1=None, W2=None, b2=None):
    # size_matrix only contributes its shape in the reference (ones_like);
    # its values are unused.
    full, _ = run_sharded(representation, W1, b1, W2, b2)
    return full
